# revision 9
# baseline (speedup 1.0000x reference)
"""Multi-head causal attention (B=128, T=256, C=384, H=6, hs=64) on 8 TRN2 cores.

Sharding: data-parallel over batch B (16 batches per core). Each core runs an
identical Bass/Tile program over its shard; host gathers the 8 output shards.

Per-core design notes (v2):
  - Everything fp16 on the PE (x, Wqkv, Wp cast on host; C^-0.5 folded into
    Wq). fp16 weights get automatic fast-weight-load (FWL) on LDWEIGHTS.
  - Batches processed in PAIRS: QT/KT projections stream both batches' x^T
    through one weight load (N=512 matmuls), 9 MMs per projection per pair.
  - S^T orientation per head pair in ONE 2-bank PSUM tile:
      bank0 = [S1_A(128) | S0_A(256)], bank1 = [S1_B(128) | S0_B(256)]
    -> one exp (2-seg AP, 768 elems) and one mask multiply (2-seg AP with
    free-dim-broadcast mask) per head pair instead of per head.
  - U per head merged to 2 matmuls: v0 @ E0 (N=256 covers both t-blocks'
    first contribution), then v1 @ E1 accumulated into the upper half.
  - V+ = [1 | 0pad | V_h] per head; U+ row 0 = softmax denominator l[t]
    (custom-DVE reciprocal reads require partition offset 0).
  - Normalization: reciprocal_approx_fast on DVE, partition_broadcast on
    GpSimd, fused evacuate+normalize tensor_mul on DVE.
  - Output projection per batch with bias via K=1 ones matmul; y evacuated
    fp16 on ACT; y DMA'd fp16, host casts to fp32.
  - Engine budget per batch (target): PE ~6us, ACT ~5.7us (exp + qkt/v/y
    copies), DVE ~5.4us (att-norm, recip, mask), GpSimd ~3.6us (broadcast).
"""
import numpy as np

B, T, C = 128, 256, 384
H, HS = 6, 64
D = H * HS  # 384
NCORES = 8
BS = B // NCORES  # 16 batches per core
NP = BS // 2  # 8 batch pairs per core
KC = C // 128  # 3 contraction chunks
MC = D // 128  # 3 output chunks
VW = 128  # per head: [ones(1) | zeros(63) | V_h(64)]

_CACHE = {}


def _build_program():
    import concourse.bacc as bacc
    import concourse.mybir as mybir
    import concourse.tile as tile

    f32 = mybir.dt.float32
    f16 = mybir.dt.float16
    Exp = mybir.ActivationFunctionType.Exp

    nc = bacc.Bacc("TRN2", target_bir_lowering=False, debug=False)

    xt_d = nc.dram_tensor("xt", [BS, C, T], f16, kind="ExternalInput").ap()
    wqkv_d = nc.dram_tensor("wqkv", [3, C, D], f16, kind="ExternalInput").ap()
    wp_d = nc.dram_tensor("wp", [D, C], f16, kind="ExternalInput").ap()
    bias_d = nc.dram_tensor("bias", [1, C], f16, kind="ExternalInput").ap()
    mask_d = nc.dram_tensor("mask", [128, 256], f16, kind="ExternalInput").ap()
    y_d = nc.dram_tensor("y", [BS, T, C], f16, kind="ExternalOutput").ap()

    with tile.TileContext(nc) as tc:
        with (
            tc.tile_pool(name="const", bufs=1) as cpool,
            tc.tile_pool(name="xt", bufs=3) as xpool,
            tc.tile_pool(name="qkt", bufs=3) as qkpool,
            tc.tile_pool(name="v", bufs=3) as vpool,
            tc.tile_pool(name="e", bufs=4) as epool,
            tc.tile_pool(name="r", bufs=3) as rpool,
            tc.tile_pool(name="att", bufs=3) as apool,
            tc.tile_pool(name="y", bufs=2) as ypool,
            tc.tile_pool(name="ps_big", bufs=2, space="PSUM") as ps_big,
            tc.tile_pool(name="ps_s", bufs=1, space="PSUM") as ps_s_pool,
            tc.tile_pool(name="ps_u", bufs=2, space="PSUM") as ps_u_pool,
        ):
            # ---- static tiles ----
            wqkv_sb = cpool.tile([128, 3 * KC * D], f16, tag="wqkv")
            wp_sb = cpool.tile([128, MC * C], f16, tag="wp")
            bias_row = cpool.tile([1, C], f16, tag="bias_row")
            mask_sb = cpool.tile([128, 256], f16, tag="mask")
            ones_row = cpool.tile([1, 128], f16, tag="ones_row")
            nc.gpsimd.memset(ones_row[:], 1.0)

            def emit_w_dma(w, k=None, eng=None):
                eng = eng or nc.sync
                if k is None:
                    eng.dma_start(
                        wqkv_sb[:, w * KC * D : (w + 1) * KC * D]
                        .rearrange("p (k d) -> p k d", k=KC),
                        wqkv_d[w].rearrange("(k p) d -> p k d", p=128),
                    )
                else:
                    eng.dma_start(
                        wqkv_sb[:, (w * KC + k) * D : (w * KC + k + 1) * D],
                        wqkv_d[w, k * 128 : (k + 1) * 128, :],
                    )

            def emit_aux_dmas():
                nc.gpsimd.dma_start(mask_sb[:], mask_d)
                nc.scalar.dma_start(
                    wp_sb[:].rearrange("p (m c) -> p m c", m=MC),
                    wp_d.rearrange("(m p) c -> p m c", p=128),
                )
                nc.gpsimd.dma_start(bias_row[:], bias_d)

            def wslice(w, k, lo, width):
                base = (w * KC + k) * D
                return wqkv_sb[:, base + lo : base + lo + width]

            # xt pair tile: [128, KC, 2T]; cols of each k-chunk = [b0 T | b1 T]
            def emit_xt(p, sb=None, eng=None):
                if sb is not None:  # startup fine-grained: one sub-batch
                    xt2 = st_xt[p]
                    (eng or nc.sync).dma_start(
                        xt2[:].rearrange("p (k s t) -> p k s t", k=KC, s=2)
                        [:, :, sb, :],
                        xt_d[2 * p + sb].rearrange("(k p) t -> p k t", p=128),
                    )
                    return xt2
                xt2 = xpool.tile([128, KC * 2 * T], f16, tag="xt", name=f"xt_{p}")
                for s in range(2):
                    nc.sync.dma_start(
                        xt2[:].rearrange("p (k s t) -> p k s t", k=KC, s=2)
                        [:, :, s, :],
                        xt_d[2 * p + s].rearrange("(k p) t -> p k t", p=128),
                    )
                return xt2

            st_xt = {}

            # QKT groups: g0=(Q,m0),(Q,m1); g1=(Q,m2),(K,m0); g2=(K,m1),(K,m2)
            # qkt layout: [128, 6, 2T], slot s = w*MC+m, cols [b0 T | b1 T]
            def emit_qkt_group(p, st, g):
                xt2 = st["xt"]
                if g == 0:
                    st["qkt"] = qkpool.tile(
                        [128, 6 * 2 * T], f16, tag="qkt", name=f"qkt_{p}"
                    )
                qkt = st["qkt"]
                ps = ps_big.tile([128, 1024], f32, tag="ps_big", name=f"psqk_{p}_{g}")
                for j in range(2):
                    slot = 2 * g + j
                    w, m = slot // MC, slot % MC
                    for k in range(KC):
                        nc.tensor.matmul(
                            ps[:, j * 512 : (j + 1) * 512],
                            wslice(w, k, m * 128, 128),
                            xt2[:, k * 512 : (k + 1) * 512],
                            start=(k == 0),
                            stop=(k == KC - 1),
                            skip_group_check=True,
                        )
                nc.scalar.copy(
                    qkt[:, 2 * g * 512 : (2 * g + 2) * 512], ps[:, 0:1024]
                )

            def emit_v(p, st, sb):
                xt2 = st["xt"]
                key = ("v", sb)
                v_sb = vpool.tile(
                    [128, 2 * H * VW], f16, tag="v", name=f"v_{p}_{sb}"
                )
                psv = ps_big.tile([128, 1024], f32, tag="ps_big", name=f"psv_{p}_{sb}")
                for tb in range(2):
                    for k in range(KC):
                        nc.tensor.matmul(
                            psv[:, tb * 512 : tb * 512 + D],
                            xt2[:, k * 512 + sb * T + tb * 128 :
                                k * 512 + sb * T + tb * 128 + 128],
                            wslice(2, k, 0, D),
                            start=(k == 0),
                            stop=(k == KC - 1),
                        )
                v4 = v_sb[:].rearrange("p (tb h c) -> p tb h c", tb=2, h=H)
                psv3 = (
                    psv[:].rearrange("p (tb c) -> p tb c", tb=2)[:, :, 0:D]
                    .rearrange("p tb (h c) -> p tb h c", h=H)
                )
                nc.scalar.copy(v4[:, 0, :, 64:VW], psv3[:, 0])
                nc.scalar.copy(v4[:, 1, :, 64:VW], psv3[:, 1])
                st[key] = v_sb

            # S for head pair (2m, 2m+1) in one 2-bank psum tile:
            #   bank0 = [S1_A(0:128) | S0_A(128:384)], bank1 = same for B @512
            # then ONE exp -> e [128, 768] = [E1_A|E0_A|E1_B|E0_B], ONE mask.
            def emit_S(b, st, m):
                p, sb = b // 2, b % 2
                qkt = st["qkt"]
                if m == 0:
                    st[("att", sb)] = [
                        apool.tile([128, T], f16, tag=f"att{mm}", name=f"att{mm}_{b}")
                        for mm in range(MC)
                    ]
                ps_s = ps_s_pool.tile(
                    [128, 1024], f32, tag="ps_s", name=f"ps_s_{b}_{m}"
                )
                e = epool.tile([128, 768], f16, tag="e", name=f"e_{b}_{m}")
                for hp in range(2):
                    off = hp * HS
                    q_ap = qkt[off : off + HS, m * 512 + sb * T : m * 512 + (sb + 1) * T]
                    k_ap = qkt[off : off + HS,
                               (MC + m) * 512 + sb * T : (MC + m) * 512 + (sb + 1) * T]
                    base = hp * 512
                    eb = hp * 384
                    nc.tensor.matmul(
                        ps_s[:, base : base + 128],
                        k_ap[:, 128:256],
                        q_ap[:, 128:256],
                        start=True,
                        stop=True,
                        skip_group_check=True,
                    )
                    nc.tensor.matmul(
                        ps_s[:, base + 128 : base + 384],
                        k_ap[:, 0:128],
                        q_ap,
                        start=True,
                        stop=True,
                        skip_group_check=True,
                    )
                    # flat-AP exp + mask per head (starts as soon as this
                    # head's bank is drained; keeps the e->U chain short)
                    nc.scalar.activation(
                        e[:, eb : eb + 384], ps_s[:, base : base + 384], Exp
                    )
                    nc.vector.tensor_mul(
                        e[:, eb : eb + 256], e[:, eb : eb + 256], mask_sb[:]
                    )
                st[("e", sb, m)] = e

            def emit_U(b, st, m):
                p, sb = b // 2, b % 2
                e = st.pop(("e", sb, m))
                att = st[("att", sb)]
                v_sb = st[("v", sb)]
                v4 = v_sb[:].rearrange("p (tb h c) -> p tb h c", tb=2, h=H)
                ps_u = ps_u_pool.tile(
                    [VW, 2 * T], f32, tag="ps_u", name=f"ps_u_{b}_{m}"
                )
                for hp in range(2):
                    h = 2 * m + hp
                    base = hp * T
                    eb = hp * 384
                    nc.tensor.matmul(
                        ps_u[:, base : base + 256],
                        v4[:, 0, h],
                        e[:, eb + 128 : eb + 384],
                        start=True,
                        stop=False,
                        skip_group_check=True,
                    )
                    nc.tensor.matmul(
                        ps_u[:, base + 128 : base + 256],
                        v4[:, 1, h],
                        e[:, eb : eb + 128],
                        start=False,
                        stop=True,
                        skip_group_check=True,
                    )
                rl = rpool.tile([1, 2 * T], f32, tag="rl", name=f"rl_{b}_{m}")
                nc.vector.reciprocal_approx_fast(rl[:], ps_u[0:1, :])
                rb = rpool.tile([HS, 2 * T], f32, tag="rb", name=f"rb_{b}_{m}")
                nc.gpsimd.partition_broadcast(rb[:], rl[:])
                for hp in range(2):
                    off = hp * HS
                    nc.vector.tensor_mul(
                        att[m][off : off + HS, :],
                        ps_u[64 : 64 + HS, hp * T : (hp + 1) * T],
                        rb[:, hp * T : (hp + 1) * T],
                    )

            def emit_proj(b, st):
                sb = b % 2
                att = st.pop(("att", sb))
                ps_y = ps_big.tile([128, 1024], f32, tag="ps_big", name=f"ps_y_{b}")
                for tb in range(2):
                    base = tb * 512
                    nc.tensor.matmul(
                        ps_y[:, base : base + C],
                        ones_row[:],
                        bias_row[:],
                        start=True,
                        stop=False,
                        skip_group_check=True,
                    )
                    for mm in range(MC):
                        nc.tensor.matmul(
                            ps_y[:, base : base + C],
                            att[mm][:, tb * 128 : (tb + 1) * 128],
                            wp_sb[:, mm * C : (mm + 1) * C],
                            start=False,
                            stop=(mm == MC - 1),
                            skip_group_check=True,
                        )
                y_sb = ypool.tile([128, 2 * C], f16, tag="y", name=f"y_{b}")
                nc.scalar.copy(
                    y_sb[:].rearrange("p (tb c) -> p tb c", tb=2),
                    ps_y[:].rearrange("p (tb c) -> p tb c", tb=2)[:, :, 0:C],
                )
                nc.sync.dma_start(
                    y_d[b].rearrange("(tb p) c -> p tb c", p=128),
                    y_sb[:].rearrange("p (tb c) -> p tb c", tb=2),
                )

            # ---- pre-zero the V+ pool buffers (static ones/pad regions) ----
            for i in range(3):
                v_init = vpool.tile([128, 2 * H * VW], f16, tag="v", name=f"vz_{i}")
                v4i = v_init[:].rearrange("p (tb h c) -> p tb h c", tb=2, h=H)
                nc.gpsimd.memset(v4i[:, :, :, 0:1], 1.0)
                nc.gpsimd.memset(v4i[:, :, :, 1:64], 0.0)

            # ---- startup: DMAs for pair 0/1 + weights, spread across the
            # sync / scalar / gpsimd DMA queues so the prefetch parallelizes.
            state = {}
            st_xt[0] = xpool.tile([128, KC * 2 * T], f16, tag="xt", name="xt_0")
            emit_xt(0, 0, eng=nc.sync)
            for k in range(KC):
                emit_w_dma(0, k, eng=nc.scalar)
            emit_xt(0, 1, eng=nc.gpsimd)
            state[0] = {"xt": st_xt[0]}
            emit_w_dma(1, eng=nc.gpsimd)
            state[1] = {"xt": emit_xt(1)}
            emit_w_dma(2, eng=nc.scalar)
            emit_aux_dmas()
            for g in range(3):
                emit_qkt_group(0, state[0], g)
            emit_v(0, state[0], 0)
            emit_v(0, state[0], 1)

            # ---- main loop over pairs ----
            # Per pair: 6 m-slots (2 sub-batches x 3 m). One PE filler per
            # slot between the exp/mask chain and the U matmuls:
            #   slot0: qkt-g0(p+1) + proj(b1 of p-1)   slot3: proj(b0)
            #   slot1: qkt-g1(p+1) [+ xt DMA p+2]      slot4: v0(p+1)
            #   slot2: qkt-g2(p+1)                     slot5: v1(p+1)
            pending_proj = None
            for p in range(NP):
                st = state[p]
                nxt = state.get(p + 1)
                for sb in range(2):
                    b = 2 * p + sb
                    for m in range(MC):
                        slot = sb * MC + m
                        emit_S(b, st, m)
                        if slot == 1 and p + 2 < NP:
                            state[p + 2] = {"xt": emit_xt(p + 2)}
                        if nxt is not None and slot < 3:
                            emit_qkt_group(p + 1, nxt, slot)
                        if slot == 0 and pending_proj is not None:
                            pending_proj()
                            pending_proj = None
                        if slot == 3 and pending_proj is not None:
                            pending_proj()
                            pending_proj = None
                        if nxt is not None and 4 <= slot:
                            emit_v(p + 1, nxt, slot - 4)
                        emit_U(b, st, m)
                    pb, pst = b, st
                    pending_proj = lambda pb=pb, pst=pst: emit_proj(pb, pst)
                state.pop(p)
            pending_proj()

    nc.compile()
    return nc


def kernel(x, Wk, Wq, Wv, Wp, bp):
    from concourse import bass_utils

    if "nc" not in _CACHE:
        _CACHE["nc"] = _build_program()
    nc = _CACHE["nc"]

    x = np.asarray(x, dtype=np.float32)
    scale = np.float32(C) ** np.float32(-0.5)
    wqkv = np.stack(
        [
            np.asarray(Wq, dtype=np.float32) * scale,
            np.asarray(Wk, dtype=np.float32),
            np.asarray(Wv, dtype=np.float32),
        ]
    ).astype(np.float16)
    wqkv = np.ascontiguousarray(wqkv)
    wp = np.ascontiguousarray(np.asarray(Wp, dtype=np.float16))
    bias = np.asarray(bp, dtype=np.float16).reshape(1, C)
    ii, jj = np.meshgrid(np.arange(128), np.arange(128), indexing="ij")
    tri = (ii <= jj).astype(np.float16)
    mask = np.ascontiguousarray(np.concatenate([tri, tri], axis=1))  # [128, 256]

    in_maps = []
    for c in range(NCORES):
        shard = x[c * BS : (c + 1) * BS]  # [BS, T, C]
        xt = np.ascontiguousarray(
            shard.transpose(0, 2, 1).astype(np.float16)
        )  # [BS, C, T]
        in_maps.append(
            {"xt": xt, "wqkv": wqkv, "wp": wp, "bias": bias, "mask": mask}
        )

    global _last_in_maps
    _last_in_maps = in_maps
    res = bass_utils.run_bass_kernel_spmd(nc, in_maps, core_ids=list(range(NCORES)))
    out = np.concatenate([r["y"] for r in res.results], axis=0)
    return out.astype(np.float32)
